# revision 10
# baseline (speedup 1.0000x reference)
"""Trainium2 Bass kernel for a 2-layer GCN over 2048 independent 25-node
KNN subgraphs (gnn_message_passing).

Strategy (v2):
  - Each 25-node subgraph is independent -> the sparse aggregation is a
    dense per-graph 25x25 matmul. Host packs the normalized adjacency
    (stored as AT[s,t]) into block-diagonal 125x128 tiles (5 graphs per
    tile) so the PE contracts over 125 partitions.
  - Aggregate FIRST: relu(A @ x @ W0) == relu((A @ x) @ W0).
      MM1: axT[fi,t] = sum_s x[s,fi] * AT[s,t]   (lhsT=x tile, mv=128)
      MM2: h1[t,fo]  = sum_fi axT[fi,t] * W0[fi,fo] (lhsT=axT, mv=256)
      MM3/4: p2T[fo,c] = sum_s h1[s,fo] * ATc[s,c]  (lhsT=h1 chunk, mv=5)
    All stationaries are 128 cols -> FWL fast weight load (bf16).
  - MM3/4 accumulate straight into two persistent PSUM banks (disjoint
    5-col slices per tile): no per-tile PSUM evacuation for layer 2.
  - PSUM evacuations are batched over BT=4 tiles (one DVE cast for axT,
    one ACT relu for h1) to amortize the per-op fixed cost.
  - Software-pipelined emission keeps the PE stream dense.
  - Inputs stream in bf16 via both HWDGE queues (sync + scalar) in
    chunks, overlapped with compute.
  - Data parallel over 8 cores: 256 graphs (52 tiles) per core.
"""

import os
import sys

import ml_dtypes
import numpy as np

for _p in ("/opt/trn_rl_repo", "/opt/trn_rl_repo/concourse"):
    if _p not in sys.path:
        sys.path.insert(0, _p)

import concourse.bass as bass
import concourse.tile as tile
from concourse import bacc, mybir
from concourse.bass_utils import run_bass_kernel_spmd

NCORES = 8
B = 2048            # graphs
K = 25              # nodes per graph
N = B * K           # 51200
GPC = B // NCORES   # 256 graphs per core
G = 5               # graphs packed per PE tile
P = G * K           # 125 partitions used per tile
NT = (GPC + G - 1) // G   # 52 tiles per core (last tile: 1 real graph)
SLOTS = NT * G      # 260 graph slots per core
AW = 128            # AT tile col width (125 block cols + 3 zero pad)
F0 = 128            # input features
F1 = 256            # hidden features
BT = 4              # tiles per pipeline batch (PSUM-evacuation batch)
NB = NT // BT       # 13 batches
# DMA chunk tile boundaries: all big inputs ride SWDGE (gpsimd) -- the
# only queue that sustains ~200GB/s on partition-shaped transfers. The
# first chunk is small so compute starts as early as SWDGE allows.
CHB = [0, 13, 32, 52]
NCH = len(CHB) - 1

_f32 = mybir.dt.float32
_bf16 = mybir.dt.bfloat16

_compiled = {}


def _build_nc():
    nc = bacc.Bacc("TRN2", target_bir_lowering=False, debug=False,
                   num_devices=NCORES)

    x_d = nc.dram_tensor("x", [P, NT * F0], _bf16, kind="ExternalInput")
    at_d = nc.dram_tensor("at", [P, NT * AW], _bf16, kind="ExternalInput")
    atc_d = nc.dram_tensor("atc", [P, NT * G], _bf16, kind="ExternalInput")
    w0_d = nc.dram_tensor("w0", [F0, F1], _bf16, kind="ExternalInput")
    w1_d = nc.dram_tensor("w1", [128, 2 * F1], _bf16, kind="ExternalInput")
    wl_d = nc.dram_tensor("wl", [128, 2], _bf16, kind="ExternalInput")
    out_d = nc.dram_tensor("out", [1, SLOTS], _f32, kind="ExternalOutput")

    relu = mybir.ActivationFunctionType.Relu

    with tile.TileContext(nc) as tc:
        with (
            tc.tile_pool(name="const", bufs=1) as cpool,
            tc.tile_pool(name="axp", bufs=2) as axp,
            tc.tile_pool(name="h1p", bufs=2) as h1p,
            tc.tile_pool(name="ps_ax", bufs=2, space=bass.MemorySpace.PSUM) as ps_ax,
            tc.tile_pool(name="ps_h1", bufs=2, space=bass.MemorySpace.PSUM) as ps_h1,
            tc.tile_pool(name="ps_p2", bufs=1, space=bass.MemorySpace.PSUM) as ps_p2,
        ):
            # ---- resident constants ----
            w0 = cpool.tile([F0, F1], _bf16, tag="w0")
            wl = cpool.tile([128, 2], _bf16, tag="wl")
            w1 = cpool.tile([128, 2 * F1], _bf16, tag="w1")
            atc_all = cpool.tile([P, NT, G], _bf16, tag="atc")
            x_ch = [cpool.tile([P, CHB[c + 1] - CHB[c], F0], _bf16,
                                tag=f"x{c}", name=f"x_ch{c}")
                    for c in range(NCH)]
            at_ch = [cpool.tile([P, CHB[c + 1] - CHB[c], AW], _bf16,
                                 tag=f"a{c}", name=f"at_ch{c}")
                     for c in range(NCH)]

            # All streaming inputs via SWDGE (gpsimd): HWDGE only engages
            # ~5 SDMA engines on partition-shaped transfers (~65GB/s) and
            # starves completely once SWDGE packets flood the shared
            # engines; SWDGE sustains ~200GB/s. Consts ride the sync
            # HWDGE queue (done before the flood).
            nc.sync.dma_start(w0[:], w0_d[:])
            nc.sync.dma_start(atc_all[:], atc_d[:])
            for c in range(NCH):
                nc.gpsimd.dma_start(at_ch[c][:],
                                    at_d[:, CHB[c] * AW:CHB[c + 1] * AW])
                nc.gpsimd.dma_start(x_ch[c][:],
                                    x_d[:, CHB[c] * F0:CHB[c + 1] * F0])
            nc.sync.dma_start(w1[:], w1_d[:])
            nc.sync.dma_start(wl[:], wl_d[:])

            # layer-2 aggregated centers, fo-chunk-major, accumulated in
            # PSUM across the whole loop (disjoint 5-col slices per tile)
            p2_ps = [ps_p2.tile([128, SLOTS], _f32, tag=f"p2{c}", name=f"p2_ps{c}")
                     for c in range(2)]

            # ---- software-pipelined per-batch stages ----
            def S0(b):                       # agg1: axT = (A @ x).T
                ps = ps_ax.tile([128, BT, F0], _f32, tag="ax")
                for j in range(BT):
                    t = b * BT + j
                    c = next(i for i in range(NCH) if CHB[i + 1] > t)
                    nc.tensor.matmul(ps[:, j, :], x_ch[c][:, t - CHB[c], :],
                                     at_ch[c][:, t - CHB[c], :],
                                     start=True, stop=True)
                return ps

            def S1(b, ps):                   # evacuate axT as bf16
                sb = axp.tile([128, BT, F0], _bf16, tag="axs")
                nc.vector.tensor_copy(sb[:], ps[:])
                return sb

            def S2(b, sb):                   # transform: h1 = axT.T @ W0
                ps = ps_h1.tile([128, BT, F1], _f32, tag="h1")
                for j in range(BT):
                    nc.tensor.matmul(ps[:, j, :], sb[:, j, :], w0[:],
                                     start=True, stop=True)
                return ps

            def S3(b, ps):                   # relu -> bf16 SBUF
                sb = h1p.tile([128, BT, F1], _bf16, tag="h1s")
                nc.scalar.activation(sb[:], ps[:], relu)
                return sb

            def S4(b, sb):                   # agg2 centers -> p2 PSUM
                for j in range(BT):
                    t = b * BT + j
                    for cc in range(2):
                        nc.tensor.matmul(
                            p2_ps[cc][:, t * G:(t + 1) * G],
                            sb[0:P, j, cc * 128:(cc + 1) * 128],
                            atc_all[:, t, :],
                            start=True, stop=True)

            ax_ps, ax_sb, h1_ps, h1_sb = {}, {}, {}, {}
            for k in range(NB + 2):
                if k < NB:
                    ax_ps[k] = S0(k)
                    ax_sb[k] = S1(k, ax_ps[k])
                if 1 <= k <= NB:
                    h1_ps[k - 1] = S2(k - 1, ax_sb[k - 1])
                    h1_sb[k - 1] = S3(k - 1, h1_ps[k - 1])
                if k >= 2:
                    S4(k - 2, h1_sb[k - 2])

            # ---- p2 PSUM -> SBUF (bf16) ----
            p2a = cpool.tile([128, 2, SLOTS], _bf16, tag="p2a")
            for cc in range(2):
                nc.vector.tensor_copy(p2a[:, cc, :], p2_ps[cc][:])

            # ---- W1 transform over all centers (weight stationary) ----
            h3_sb = cpool.tile([128, 2, SLOTS], _bf16, tag="h3")
            for fo in range(2):
                h3_ps = ps_ax.tile([128, SLOTS], _f32, tag="ax")
                for fi in range(2):
                    nc.tensor.matmul(
                        h3_ps[:],
                        w1[:, fi * F1 + fo * 128:fi * F1 + fo * 128 + 128],
                        p2a[:, fi, :],
                        start=(fi == 0), stop=(fi == 1))
                nc.scalar.activation(h3_sb[:, fo, :], h3_ps[:], relu)

            # ---- out = relu(h3).T @ Wlin ----
            out_ps = ps_h1.tile([1, SLOTS], _f32, tag="h1")
            for fo in range(2):
                nc.tensor.matmul(out_ps[:], wl[:, fo:fo + 1], h3_sb[:, fo, :],
                                 start=(fo == 0), stop=(fo == 1))
            out_sb = cpool.tile([1, SLOTS], _f32, tag="out")
            nc.vector.tensor_copy(out_sb[:], out_ps[:])
            nc.sync.dma_start(out_d[:], out_sb[:])

    nc.compile()
    return nc


def _get_nc():
    if "nc" not in _compiled:
        _compiled["nc"] = _build_nc()
    return _compiled["nc"]


def _host_prep(x, edge_weight, W0, W1, Wlin, edge_index):
    bf = ml_dtypes.bfloat16
    src = edge_index[0].astype(np.int64)
    tgt = edge_index[1].astype(np.int64)
    b = src // K
    sl = src - b * K
    tl = tgt - (tgt // K) * K

    # dense raw adjacency per graph, indexed [b, t, s]
    idx = (b * K + tl) * K + sl
    Araw = np.bincount(idx, weights=edge_weight.astype(np.float64),
                       minlength=B * K * K).astype(np.float32).reshape(B, K, K)
    deg = Araw.sum(axis=2)                      # weighted in-degree [B, K]
    with np.errstate(divide="ignore"):
        dinv = np.where(deg > 0, 1.0 / np.sqrt(deg), 0.0).astype(np.float32)
    An = Araw * dinv[:, :, None] * dinv[:, None, :]   # [b, t, s]
    ATn = np.ascontiguousarray(An.transpose(0, 2, 1))  # [b, s, t]

    # scatter graphs into per-core padded slots
    ATs = np.zeros((NCORES, SLOTS, K, K), np.float32)
    ATs[:, :GPC] = ATn.reshape(NCORES, GPC, K, K)
    ATs = ATs.reshape(NCORES, NT, G, K, K)

    at = np.zeros((NCORES, NT, P, AW), np.float32)
    bd = at[..., :P].reshape(NCORES, NT, G, K, G, K)
    atc = np.zeros((NCORES, NT, P, G), np.float32)
    cent = atc.reshape(NCORES, NT, G, K, G)
    for g in range(G):
        bd[:, :, g, :, g, :] = ATs[:, :, g]          # block-diagonal AT[s,t]
        cent[:, :, g, :, g] = ATs[:, :, g, :, 0]     # center (t_local=0) col
    # partition(s)-major device layout
    at = np.ascontiguousarray(at.transpose(0, 2, 1, 3).astype(bf))
    atc = np.ascontiguousarray(atc.transpose(0, 2, 1, 3).astype(bf))

    # x node-major per tile: [core, s, tile, F0]
    xp = np.zeros((NCORES, NT * P, F0), np.float32)
    xp[:, :GPC * K] = x.reshape(NCORES, GPC * K, F0)
    xp = np.ascontiguousarray(
        xp.reshape(NCORES, NT, P, F0).transpose(0, 2, 1, 3).astype(bf))

    # W1 packed: [:, fi*256 + fo*128 + c] = W1[fi*128 + r?, ...]
    w1p = np.empty((128, 2 * F1), np.float32)
    w1p[:, 0:F1] = W1[0:128, :]
    w1p[:, F1:2 * F1] = W1[128:256, :]
    w1p = np.ascontiguousarray(w1p.astype(bf))
    wl = np.ascontiguousarray(Wlin.reshape(2, 128).T.astype(bf))
    w0b = np.ascontiguousarray(W0.astype(bf))

    in_maps = []
    for c in range(NCORES):
        in_maps.append({
            "x": xp[c].reshape(P, NT * F0),
            "at": at[c].reshape(P, NT * AW),
            "atc": atc[c].reshape(P, NT * G),
            "w0": w0b,
            "w1": w1p,
            "wl": wl,
        })
    return in_maps


def _run(inputs, mode=None, trace=False):
    nc = _get_nc()
    in_maps = _host_prep(**inputs)
    res = run_bass_kernel_spmd(nc, in_maps, core_ids=list(range(NCORES)),
                               trace=trace)
    out = np.empty((B, 1), np.float32)
    for c in range(NCORES):
        out[c * GPC:(c + 1) * GPC, 0] = res.results[c]["out"][0, :GPC]
    return out, res


def kernel(**inputs):
    out, _ = _run(inputs, trace=False)
    return out


# revision 11
# speedup vs baseline: 1.0267x; 1.0267x over previous
"""Trainium2 Bass kernel for a 2-layer GCN over 2048 independent 25-node
KNN subgraphs (gnn_message_passing).

Strategy (v2):
  - Each 25-node subgraph is independent -> the sparse aggregation is a
    dense per-graph 25x25 matmul. Host packs the normalized adjacency
    (stored as AT[s,t]) into block-diagonal 125x128 tiles (5 graphs per
    tile) so the PE contracts over 125 partitions.
  - Aggregate FIRST: relu(A @ x @ W0) == relu((A @ x) @ W0).
      MM1: axT[fi,t] = sum_s x[s,fi] * AT[s,t]   (lhsT=x tile, mv=128)
      MM2: h1[t,fo]  = sum_fi axT[fi,t] * W0[fi,fo] (lhsT=axT, mv=256)
      MM3/4: p2T[fo,c] = sum_s h1[s,fo] * ATc[s,c]  (lhsT=h1 chunk, mv=5)
    All stationaries are 128 cols -> FWL fast weight load (bf16).
  - MM3/4 accumulate straight into two persistent PSUM banks (disjoint
    5-col slices per tile): no per-tile PSUM evacuation for layer 2.
  - PSUM evacuations are batched over BT=4 tiles (one DVE cast for axT,
    one ACT relu for h1) to amortize the per-op fixed cost.
  - Software-pipelined emission keeps the PE stream dense.
  - Inputs stream in bf16 via both HWDGE queues (sync + scalar) in
    chunks, overlapped with compute.
  - Data parallel over 8 cores: 256 graphs (52 tiles) per core.
"""

import os
import sys

import ml_dtypes
import numpy as np

for _p in ("/opt/trn_rl_repo", "/opt/trn_rl_repo/concourse"):
    if _p not in sys.path:
        sys.path.insert(0, _p)

import concourse.bass as bass
import concourse.tile as tile
from concourse import bacc, mybir
from concourse.bass_utils import run_bass_kernel_spmd

NCORES = 8
B = 2048            # graphs
K = 25              # nodes per graph
N = B * K           # 51200
GPC = B // NCORES   # 256 graphs per core
G = 5               # graphs packed per PE tile
P = G * K           # 125 partitions used per tile
NT = (GPC + G - 1) // G   # 52 tiles per core (last tile: 1 real graph)
SLOTS = NT * G      # 260 graph slots per core
AW = 128            # AT tile col width (125 block cols + 3 zero pad)
F0 = 128            # input features
F1 = 256            # hidden features
BT = 4              # tiles per pipeline batch (PSUM-evacuation batch)
NB = NT // BT       # 13 batches
# DMA chunk tile boundaries: all big inputs ride SWDGE (gpsimd) -- the
# only queue that sustains ~200GB/s on partition-shaped transfers. The
# first chunk is small so compute starts as early as SWDGE allows.
CHB = [0, 13, 26, 39, 52]
NCH = len(CHB) - 1

_f32 = mybir.dt.float32
_bf16 = mybir.dt.bfloat16

_compiled = {}


def _build_nc():
    nc = bacc.Bacc("TRN2", target_bir_lowering=False, debug=False,
                   num_devices=NCORES)

    x_d = nc.dram_tensor("x", [P, NT * F0], _bf16, kind="ExternalInput")
    at_d = nc.dram_tensor("at", [P, NT * AW], _bf16, kind="ExternalInput")
    atc_d = nc.dram_tensor("atc", [P, NT * G], _bf16, kind="ExternalInput")
    w0_d = nc.dram_tensor("w0", [F0, F1], _bf16, kind="ExternalInput")
    w1_d = nc.dram_tensor("w1", [128, 2 * F1], _bf16, kind="ExternalInput")
    wl_d = nc.dram_tensor("wl", [128, 2], _bf16, kind="ExternalInput")
    out_d = nc.dram_tensor("out", [1, SLOTS], _f32, kind="ExternalOutput")

    relu = mybir.ActivationFunctionType.Relu

    with tile.TileContext(nc) as tc:
        with (
            tc.tile_pool(name="const", bufs=1) as cpool,
            tc.tile_pool(name="axp", bufs=2) as axp,
            tc.tile_pool(name="h1p", bufs=2) as h1p,
            tc.tile_pool(name="ps_ax", bufs=2, space=bass.MemorySpace.PSUM) as ps_ax,
            tc.tile_pool(name="ps_h1", bufs=2, space=bass.MemorySpace.PSUM) as ps_h1,
            tc.tile_pool(name="ps_p2", bufs=1, space=bass.MemorySpace.PSUM) as ps_p2,
        ):
            # ---- resident constants ----
            w0 = cpool.tile([F0, F1], _bf16, tag="w0")
            wl = cpool.tile([128, 2], _bf16, tag="wl")
            w1 = cpool.tile([128, 2 * F1], _bf16, tag="w1")
            atc_all = cpool.tile([P, NT, G], _bf16, tag="atc")
            x_ch = [cpool.tile([P, CHB[c + 1] - CHB[c], F0], _bf16,
                                tag=f"x{c}", name=f"x_ch{c}")
                    for c in range(NCH)]
            at_ch = [cpool.tile([P, CHB[c + 1] - CHB[c], AW], _bf16,
                                 tag=f"a{c}", name=f"at_ch{c}")
                     for c in range(NCH)]

            # All streaming inputs via SWDGE (gpsimd): HWDGE only engages
            # ~5 SDMA engines on partition-shaped transfers (~65GB/s) and
            # starves completely once SWDGE packets flood the shared
            # engines; SWDGE sustains ~200GB/s. Consts ride the sync
            # HWDGE queue (done before the flood).
            nc.sync.dma_start(w0[:], w0_d[:])
            nc.sync.dma_start(atc_all[:], atc_d[:])
            for c in range(NCH):
                nc.gpsimd.dma_start(at_ch[c][:],
                                    at_d[:, CHB[c] * AW:CHB[c + 1] * AW])
                nc.gpsimd.dma_start(x_ch[c][:],
                                    x_d[:, CHB[c] * F0:CHB[c + 1] * F0])
            nc.sync.dma_start(w1[:], w1_d[:])
            nc.sync.dma_start(wl[:], wl_d[:])

            # layer-2 aggregated centers, fo-chunk-major, accumulated in
            # PSUM across the whole loop (disjoint 5-col slices per tile)
            p2_ps = [ps_p2.tile([128, SLOTS], _f32, tag=f"p2{c}", name=f"p2_ps{c}")
                     for c in range(2)]

            # ---- software-pipelined per-batch stages ----
            def S0(b):                       # agg1: axT = (A @ x).T
                ps = ps_ax.tile([128, BT, F0], _f32, tag="ax")
                for j in range(BT):
                    t = b * BT + j
                    c = next(i for i in range(NCH) if CHB[i + 1] > t)
                    nc.tensor.matmul(ps[:, j, :], x_ch[c][:, t - CHB[c], :],
                                     at_ch[c][:, t - CHB[c], :],
                                     start=True, stop=True)
                return ps

            def S1(b, ps):                   # evacuate axT as bf16
                sb = axp.tile([128, BT, F0], _bf16, tag="axs")
                nc.vector.tensor_copy(sb[:], ps[:])
                return sb

            def S2(b, sb):                   # transform: h1 = axT.T @ W0
                ps = ps_h1.tile([128, BT, F1], _f32, tag="h1")
                for j in range(BT):
                    nc.tensor.matmul(ps[:, j, :], sb[:, j, :], w0[:],
                                     start=True, stop=True)
                return ps

            def S3(b, ps):                   # relu -> bf16 SBUF
                sb = h1p.tile([128, BT, F1], _bf16, tag="h1s")
                nc.scalar.activation(sb[:], ps[:], relu)
                return sb

            def S4(b, sb):                   # agg2 centers -> p2 PSUM
                for j in range(BT):
                    t = b * BT + j
                    for cc in range(2):
                        nc.tensor.matmul(
                            p2_ps[cc][:, t * G:(t + 1) * G],
                            sb[0:P, j, cc * 128:(cc + 1) * 128],
                            atc_all[:, t, :],
                            start=True, stop=True)

            ax_ps, ax_sb, h1_ps, h1_sb = {}, {}, {}, {}
            for k in range(NB + 2):
                if k < NB:
                    ax_ps[k] = S0(k)
                    ax_sb[k] = S1(k, ax_ps[k])
                if 1 <= k <= NB:
                    h1_ps[k - 1] = S2(k - 1, ax_sb[k - 1])
                    h1_sb[k - 1] = S3(k - 1, h1_ps[k - 1])
                if k >= 2:
                    S4(k - 2, h1_sb[k - 2])

            # ---- p2 PSUM -> SBUF (bf16) ----
            p2a = cpool.tile([128, 2, SLOTS], _bf16, tag="p2a")
            for cc in range(2):
                nc.vector.tensor_copy(p2a[:, cc, :], p2_ps[cc][:])

            # ---- W1 transform over all centers (weight stationary) ----
            h3_sb = cpool.tile([128, 2, SLOTS], _bf16, tag="h3")
            for fo in range(2):
                h3_ps = ps_ax.tile([128, SLOTS], _f32, tag="ax")
                for fi in range(2):
                    nc.tensor.matmul(
                        h3_ps[:],
                        w1[:, fi * F1 + fo * 128:fi * F1 + fo * 128 + 128],
                        p2a[:, fi, :],
                        start=(fi == 0), stop=(fi == 1))
                nc.scalar.activation(h3_sb[:, fo, :], h3_ps[:], relu)

            # ---- out = relu(h3).T @ Wlin ----
            out_ps = ps_h1.tile([1, SLOTS], _f32, tag="h1")
            for fo in range(2):
                nc.tensor.matmul(out_ps[:], wl[:, fo:fo + 1], h3_sb[:, fo, :],
                                 start=(fo == 0), stop=(fo == 1))
            out_sb = cpool.tile([1, SLOTS], _f32, tag="out")
            nc.vector.tensor_copy(out_sb[:], out_ps[:])
            nc.sync.dma_start(out_d[:], out_sb[:])

    nc.compile()
    return nc


def _get_nc():
    if "nc" not in _compiled:
        _compiled["nc"] = _build_nc()
    return _compiled["nc"]


def _host_prep(x, edge_weight, W0, W1, Wlin, edge_index):
    bf = ml_dtypes.bfloat16
    src = edge_index[0].astype(np.int64)
    tgt = edge_index[1].astype(np.int64)
    b = src // K
    sl = src - b * K
    tl = tgt - (tgt // K) * K

    # dense raw adjacency per graph, indexed [b, t, s]
    idx = (b * K + tl) * K + sl
    Araw = np.bincount(idx, weights=edge_weight.astype(np.float64),
                       minlength=B * K * K).astype(np.float32).reshape(B, K, K)
    deg = Araw.sum(axis=2)                      # weighted in-degree [B, K]
    with np.errstate(divide="ignore"):
        dinv = np.where(deg > 0, 1.0 / np.sqrt(deg), 0.0).astype(np.float32)
    An = Araw * dinv[:, :, None] * dinv[:, None, :]   # [b, t, s]
    ATn = np.ascontiguousarray(An.transpose(0, 2, 1))  # [b, s, t]

    # scatter graphs into per-core padded slots
    ATs = np.zeros((NCORES, SLOTS, K, K), np.float32)
    ATs[:, :GPC] = ATn.reshape(NCORES, GPC, K, K)
    ATs = ATs.reshape(NCORES, NT, G, K, K)

    at = np.zeros((NCORES, NT, P, AW), np.float32)
    bd = at[..., :P].reshape(NCORES, NT, G, K, G, K)
    atc = np.zeros((NCORES, NT, P, G), np.float32)
    cent = atc.reshape(NCORES, NT, G, K, G)
    for g in range(G):
        bd[:, :, g, :, g, :] = ATs[:, :, g]          # block-diagonal AT[s,t]
        cent[:, :, g, :, g] = ATs[:, :, g, :, 0]     # center (t_local=0) col
    # partition(s)-major device layout
    at = np.ascontiguousarray(at.transpose(0, 2, 1, 3).astype(bf))
    atc = np.ascontiguousarray(atc.transpose(0, 2, 1, 3).astype(bf))

    # x node-major per tile: [core, s, tile, F0]
    xp = np.zeros((NCORES, NT * P, F0), np.float32)
    xp[:, :GPC * K] = x.reshape(NCORES, GPC * K, F0)
    xp = np.ascontiguousarray(
        xp.reshape(NCORES, NT, P, F0).transpose(0, 2, 1, 3).astype(bf))

    # W1 packed: [:, fi*256 + fo*128 + c] = W1[fi*128 + r?, ...]
    w1p = np.empty((128, 2 * F1), np.float32)
    w1p[:, 0:F1] = W1[0:128, :]
    w1p[:, F1:2 * F1] = W1[128:256, :]
    w1p = np.ascontiguousarray(w1p.astype(bf))
    wl = np.ascontiguousarray(Wlin.reshape(2, 128).T.astype(bf))
    w0b = np.ascontiguousarray(W0.astype(bf))

    in_maps = []
    for c in range(NCORES):
        in_maps.append({
            "x": xp[c].reshape(P, NT * F0),
            "at": at[c].reshape(P, NT * AW),
            "atc": atc[c].reshape(P, NT * G),
            "w0": w0b,
            "w1": w1p,
            "wl": wl,
        })
    return in_maps


def _run(inputs, mode=None, trace=False):
    nc = _get_nc()
    in_maps = _host_prep(**inputs)
    res = run_bass_kernel_spmd(nc, in_maps, core_ids=list(range(NCORES)),
                               trace=trace)
    out = np.empty((B, 1), np.float32)
    for c in range(NCORES):
        out[c * GPC:(c + 1) * GPC, 0] = res.results[c]["out"][0, :GPC]
    return out, res


def kernel(**inputs):
    out, _ = _run(inputs, trace=False)
    return out
